# revision 40
# baseline (speedup 1.0000x reference)
"""OCAB (overlapping cross-attention block) Trainium2 Bass kernel.

Full inputs in, full outputs out; internally shards the B*nW window axis
across 8 NeuronCores (each core owns 2 window-rows = 32 image rows, with a
4-row halo for the overlapping k/v windows).

v2 redesign vs the original baseline:
  - LayerNorm runs on DVE+ACT (bn_stats/bn_aggr, Sqrt with fused +eps bias,
    reciprocal, tensor_scalar) -- no GpSimd in the hot path.
  - V is projected into a token-major SBUF slab, then staged to a DRAM
    scratch once; each window's 5 key-chunks are gathered back with ONE
    DMA per chunk (column-major chunk layout makes the access pattern
    affine), replacing ~25 tiny SBUF-SBUF DMAs per window.
  - S^T PSUM packs all 6 heads in [128, 1536] (3 banks); one exp per chunk.
  - attn@V has no duplicate fill matmuls.
  - Windows are software-pipelined: the denominator-broadcast and
    projection matmuls of window w-1 are interleaved into window w's
    S^T stream so the PE never drains (keeps the HAM clock warm).
"""

import os
import sys
from contextlib import ExitStack

import numpy as np
import ml_dtypes

for _p in ("/opt/trn_rl_repo", "/root/.axon_site/_ro/trn_rl_repo"):
    if os.path.isdir(_p) and _p not in sys.path:
        sys.path.append(_p)

import concourse.bass as bass
import concourse.tile as tile
from concourse import bacc, mybir
from concourse.bass_utils import run_bass_kernel_spmd

BF16 = mybir.dt.bfloat16
F32 = mybir.dt.float32
F32R = mybir.dt.float32r
bfnp = ml_dtypes.bfloat16

# ---- problem constants (hardcoded per contract) ----
C = 180
NH = 6
HD = 30
WS = 16
OWS = 24
PADW = 4
H = W = 256
EPS = 1e-5
NCORES = 8

# ---- per-core slab geometry ----
RS = 40          # slab image rows (32 + 2*4 halo)
CS = 264         # slab image cols (256 + 2*4 zero pad)
TS_REAL = RS * CS          # 10560 real slab tokens
TCH = 84                   # token chunks of 128
T = TCH * 128              # 10752 padded slab tokens
NG = 21                    # 512-token groups (21*512 == 10752)
NWIN = 32                  # windows per core (2 window-rows x 16)
# column-major key chunks: chunk j covers patch cols [5j, 5j+ncol), all 24
# patch rows; within-chunk key index = 24*c_local + r.
CHUNK_NCOL = [5, 5, 5, 5, 4]
CHUNK_KC = [24 * n for n in CHUNK_NCOL]      # 120,120,120,120,96
# head -> st/es column; same-bank pairs (h0,h4)(h1,h5)(h2,h3) share a PE
# row group (A-tile rows h0/h1/h2 at 0/32/64, B-tile rows h4/h5/h3), so the
# 3-bank st can double-buffer without illegal concurrent same-bank writes.
STC = {0: 0, 4: 256, 1: 512, 5: 768, 2: 1024, 3: 1280}
HROW = {0: 0, 1: 32, 2: 64, 4: 0, 5: 32, 3: 64}

LAST_RESULTS = None
_CACHED = None


def _build_program():
    nc = bacc.Bacc("TRN2", target_bir_lowering=False)

    xs_d = nc.declare_dram_parameter("xs", [T, C], F32, isOutput=False)
    xr_d = nc.declare_dram_parameter("xr", [8192, C], F32, isOutput=False)
    wqk_d = nc.declare_dram_parameter("wqk", [181, 512], BF16, isOutput=False)
    wv_d = nc.declare_dram_parameter("wv", [181, 192], BF16, isOutput=False)
    wp_d = nc.declare_dram_parameter("wp", [192, C], BF16, isOutput=False)
    id_d = nc.declare_dram_parameter("ident", [128, 128], BF16, isOutput=False)
    e128_d = nc.declare_dram_parameter("e128", [128, 128], F32R, isOutput=False)
    e64_d = nc.declare_dram_parameter("e64", [64, 128], F32R, isOutput=False)
    ones_d = nc.declare_dram_parameter("ones", [1, T], BF16, isOutput=False)
    out_d = nc.declare_dram_parameter("out", [8192, C], F32, isOutput=True)

    with ExitStack() as ctx:
        tc = ctx.enter_context(tile.TileContext(nc))

        # ---- persistent: weights, q/k transposed slabs, V DRAM scratch ----
        wp_pool = ctx.enter_context(tc.tile_pool(name="wts", bufs=1))
        WQK0 = wp_pool.tile([128, 512], BF16, tag="wqk0")
        WQK1 = wp_pool.tile([53, 512], BF16, tag="wqk1")
        WV0 = wp_pool.tile([128, 192], BF16, tag="wv0")
        WV1 = wp_pool.tile([53, 192], BF16, tag="wv1")
        WP0 = wp_pool.tile([128, C], BF16, tag="wp0")
        WP1 = wp_pool.tile([64, C], BF16, tag="wp1")
        IDT = wp_pool.tile([128, 128], BF16, tag="id")
        E128 = wp_pool.tile([128, 128], F32R, tag="e128")
        E64 = wp_pool.tile([64, 128], F32R, tag="e64")
        EPSB = wp_pool.tile([128, 1], F32, tag="epsb")
        nc.vector.memset(EPSB[:], EPS)

        slab = ctx.enter_context(tc.tile_pool(name="slab", bufs=1))
        QS = [
            slab.tile([128, T], BF16, tag=f"qs{i}", name=f"qs{i}") for i in range(4)
        ]

        dram = ctx.enter_context(tc.tile_pool(name="dram", bufs=1, space="DRAM"))
        VD = dram.tile([T, 192], BF16, tag="vd")

        nc.sync.dma_start(WQK0[:], wqk_d[0:128, :])
        nc.sync.dma_start(WQK1[:], wqk_d[128:181, :])
        nc.sync.dma_start(WV0[:], wv_d[0:128, :])
        nc.sync.dma_start(WV1[:], wv_d[128:181, :])
        nc.sync.dma_start(WP0[:], wp_d[0:128, :])
        nc.sync.dma_start(WP1[:], wp_d[128:192, :])
        nc.sync.dma_start(IDT[:], id_d[:, :])
        nc.sync.dma_start(E128[:], e128_d[:, :])
        nc.sync.dma_start(E64[:], e64_d[:, :])

        # ================= phase 1+2: LN, transpose, projections ============
        with ExitStack() as pctx:
            slab2 = pctx.enter_context(tc.tile_pool(name="slab2", bufs=1))
            XT0 = slab2.tile([128, T], BF16, tag="xt0")   # xn^T channels 0-127
            XT1 = slab2.tile([53, T], BF16, tag="xt1")    # ch 128-179 + ones row
            VS = slab2.tile([128, TCH * 192], BF16, tag="vs")
            nc.sync.dma_start(XT1[52:53, :], ones_d[:, :])

            p_x = pctx.enter_context(tc.tile_pool(name="p_x", bufs=8))
            p_sm = pctx.enter_context(tc.tile_pool(name="p_sm", bufs=10))
            p_ps = pctx.enter_context(tc.tile_pool(name="p_ps", bufs=2, space="PSUM"))

            def emit_proj(g):
                """projections for token group g (q skips halo groups); the
                LN loop emits group g-1's projections so the PE always has
                dense N=512 streams to chew on between transposes."""
                for mc in range(4):
                    if mc < 2 and not (2 <= g <= 18):
                        continue
                    qp = p_ps.tile([128, 512], F32, tag="mm")
                    nc.tensor.matmul(
                        qp[:],
                        WQK0[:, 128 * mc : 128 * (mc + 1)],
                        XT0[:, 512 * g : 512 * (g + 1)],
                        start=True,
                        stop=False,
                    )
                    nc.tensor.matmul(
                        qp[:],
                        WQK1[:, 128 * mc : 128 * (mc + 1)],
                        XT1[:, 512 * g : 512 * (g + 1)],
                        start=False,
                        stop=True,
                    )
                    if (g + mc) % 2 == 0:
                        nc.vector.tensor_copy(
                            QS[mc][:, 512 * g : 512 * (g + 1)], qp[:]
                        )
                    else:
                        nc.scalar.copy(QS[mc][:, 512 * g : 512 * (g + 1)], qp[:])
                for pair in (2 * g, 2 * g + 1):
                    vp = p_ps.tile([128, 384], F32, tag="vmm")
                    for j in range(2):
                        tch = 2 * pair + j
                        nc.tensor.matmul(
                            vp[:, 192 * j : 192 * (j + 1)],
                            XT0[:, 128 * tch : 128 * (tch + 1)],
                            WV0[:],
                            start=True,
                            stop=False,
                        )
                        nc.tensor.matmul(
                            vp[:, 192 * j : 192 * (j + 1)],
                            XT1[:, 128 * tch : 128 * (tch + 1)],
                            WV1[:],
                            start=False,
                            stop=True,
                        )
                    if pair % 2 == 0:
                        nc.vector.tensor_copy(
                            VS[:, 384 * pair : 384 * (pair + 1)], vp[:]
                        )
                    else:
                        nc.scalar.copy(VS[:, 384 * pair : 384 * (pair + 1)], vp[:])
                # stage this group's V to DRAM (token-major) while later
                # groups are still projecting
                nc.sync.dma_start(
                    VD[512 * g : 512 * (g + 1), :].rearrange(
                        "(b p) v -> p b v", p=128
                    ),
                    VS[:, 768 * g : 768 * (g + 1)].rearrange(
                        "p (b v) -> p b v", v=192
                    ),
                )

            for g in range(TCH // 4):          # 21 groups of 4 token chunks
                tp = p_ps.tile([128, 512], BF16, tag="tp")
                tp2 = p_ps.tile([52, 512], BF16, tag="tp2")
                for j2 in range(2):
                    xt2 = p_x.tile([128, 360], F32, tag="x")
                    t2 = 2 * g + j2
                    nc.sync.dma_start(
                        xt2[:].rearrange("p (b c) -> p b c", c=C),
                        xs_d[256 * t2 : 256 * (t2 + 1), :].rearrange(
                            "(b p) c -> p b c", p=128
                        ),
                    )
                    for jh in range(2):
                        j = 2 * j2 + jh
                        xt = xt2[:, C * jh : C * (jh + 1)]
                        stats = p_sm.tile([128, 6], F32, tag="st")
                        aggr = p_sm.tile([128, 2], F32, tag="ag")
                        nc.vector.bn_stats(stats[:], xt)
                        nc.vector.bn_aggr(aggr[:], stats[:])
                        sd = p_sm.tile([128, 1], F32, tag="sd")
                        nc.scalar.activation(
                            sd[:], aggr[:, 1:2], mybir.ActivationFunctionType.Sqrt,
                            bias=EPSB[:, 0:1],
                        )
                        rstd = p_sm.tile([128, 1], F32, tag="rstd")
                        nc.vector.reciprocal(rstd[:], sd[:])
                        xn = p_x.tile([128, C], BF16, tag="xn")
                        nc.vector.tensor_scalar(
                            xn[:],
                            xt,
                            aggr[:, 0:1],
                            rstd[:],
                            op0=mybir.AluOpType.subtract,
                            op1=mybir.AluOpType.mult,
                        )
                        nc.tensor.transpose(
                            tp[:, 128 * j : 128 * (j + 1)], xn[:, 0:128], IDT[:]
                        )
                        nc.tensor.transpose(
                            tp2[:, 128 * j : 128 * (j + 1)], xn[:, 128:180], IDT[:]
                        )
                if g % 2 == 0:
                    nc.scalar.copy(XT0[:, 512 * g : 512 * (g + 1)], tp[:])
                    nc.vector.tensor_copy(XT1[0:52, 512 * g : 512 * (g + 1)], tp2[:])
                else:
                    nc.vector.tensor_copy(XT0[:, 512 * g : 512 * (g + 1)], tp[:])
                    nc.scalar.copy(XT1[0:52, 512 * g : 512 * (g + 1)], tp2[:])
                if g > 0:
                    emit_proj(g - 1)
            emit_proj(TCH // 4 - 1)

        # ================= phase 3: windowed attention =======================
        with ExitStack() as actx:
            a_st = actx.enter_context(tc.tile_pool(name="a_st", bufs=2, space="PSUM"))
            a_av = actx.enter_context(tc.tile_pool(name="a_av", bufs=1, space="PSUM"))
            a_xp = actx.enter_context(tc.tile_pool(name="a_xp", bufs=1, space="PSUM"))
            a_es = actx.enter_context(tc.tile_pool(name="a_es", bufs=11))
            a_vw = actx.enter_context(tc.tile_pool(name="a_vw", bufs=16))
            a_kw = actx.enter_context(tc.tile_pool(name="a_kw", bufs=2))
            a_sb = actx.enter_context(tc.tile_pool(name="a_sb", bufs=2))

            # r-major per-window views of the q slabs (for the moving q operand)
            qs_pat = [
                QS[i][:, 0:TS_REAL].rearrange("p (r c) -> p r c", c=CS)
                for i in range(4)
            ]
            # c-major views of the k slabs (for the kw materialization)
            qs_cpat = [
                QS[i][:, 0:TS_REAL].rearrange("p (r c) -> p c r", c=CS)
                for i in range(4)
            ]
            # c-major view of the V DRAM scratch
            vd_c = VD[0:TS_REAL, :].rearrange("(r c) v -> c r v", c=CS)

            xr_pat = xr_d[:, :].rearrange("(r c) d -> r c d", c=W)
            out_pat = out_d[:, :].rearrange("(r c) d -> r c d", c=W)

            def emit_vw(w):
                """one DMA per key chunk: [ncol, 24, 192] dram -> [kc, 192]"""
                wrl, wc = w // 16, w % 16
                r0, c0 = WS * wrl, WS * wc
                tiles = []
                for j in range(5):
                    ncol, kc = CHUNK_NCOL[j], CHUNK_KC[j]
                    vw = a_vw.tile([128, 192], BF16, tag="vw", name=f"vw{w}_{j}")
                    nc.sync.dma_start(
                        vw[0:kc, :],
                        vd_c[c0 + 5 * j : c0 + 5 * j + ncol, r0 : r0 + 24, :],
                    )
                    tiles.append(vw)
                return tiles

            def emit_kw(w):
                """k^T/q window tiles packed as head triples: A = (h0,h1,h2)
                at rows 0/32/64, B = (h4,h5,h3) at rows 0/32/64 (c-major key
                order, col = 24c+r).  Copies stay wide and base-partition
                legal: [0:96], [0:64], [64:96]."""
                wrl, wc = w // 16, w % 16
                r0, c0 = WS * wrl, WS * wc
                kwA = a_kw.tile([96, 576], BF16, tag="kwA", name=f"kwA_{w}")
                kwB = a_kw.tile([96, 576], BF16, tag="kwB", name=f"kwB_{w}")
                qwA = a_kw.tile([96, 256], BF16, tag="qwA", name=f"qwA_{w}")
                qwB = a_kw.tile([96, 256], BF16, tag="qwB", name=f"qwB_{w}")
                kA = kwA[:].rearrange("p (c r) -> p c r", r=OWS)
                kB = kwB[:].rearrange("p (c r) -> p c r", r=OWS)
                ksl = (slice(c0, c0 + OWS), slice(r0, r0 + OWS))
                nc.vector.tensor_copy(kA, qs_cpat[2][0:96, ksl[0], ksl[1]])
                nc.vector.tensor_copy(kB[0:64], qs_cpat[3][0:64, ksl[0], ksl[1]])
                nc.vector.tensor_copy(kB[64:96], qs_cpat[2][96:128, ksl[0], ksl[1]])
                qA = qwA[:].rearrange("p (a b) -> p a b", b=WS)
                qB = qwB[:].rearrange("p (a b) -> p a b", b=WS)
                qsl = (
                    slice(PADW + WS * wrl, PADW + WS * wrl + WS),
                    slice(PADW + c0, PADW + c0 + WS),
                )
                nc.vector.tensor_copy(qA, qs_pat[0][0:96, qsl[0], qsl[1]])
                nc.vector.tensor_copy(qB[0:64], qs_pat[1][0:64, qsl[0], qsl[1]])
                nc.vector.tensor_copy(qB[64:96], qs_pat[0][96:128, qsl[0], qsl[1]])
                return (kwA, kwB), (qwA, qwB)

            def emit_xres(w):
                wrl, wc = w // 16, w % 16
                c0 = WS * wc
                xres = a_sb.tile([128, 360], F32, tag="xres", name=f"xres{w}", bufs=3)
                for qc in range(2):
                    rq = WS * wrl + 8 * qc
                    nc.gpsimd.dma_start(
                        xres[:, 180 * qc : 180 * qc + 180],
                        xr_pat[rq : rq + 8, c0 : c0 + WS, :],
                    )
                return xres

            def av_group(pv, h):
                """one contiguous 5-matmul PSUM accumulation group for head h
                of the previous window (groups must never interleave)."""
                colp, colf = 32 * (h % 4), 256 * (h // 4)
                for j in range(5):
                    kc = CHUNK_KC[j]
                    nc.tensor.matmul(
                        pv["av"][colp : colp + 32, colf : colf + 256],
                        pv["vw"][j][0:kc, 32 * h : 32 * h + 32],
                        pv["es"][j][0:kc, STC[h] : STC[h] + 256],
                        start=(j == 0),
                        stop=(j == 4),
                        tile_position=(0, colp),
                    )

            def filler(pv, j):
                """previous-window work dropped into round j's exp shadow"""
                if pv is None:
                    return
                w = pv["w"]
                if j == 0:
                    pv["av"] = a_av.tile([128, 512], F32, tag="av", name=f"av{w}")
                    av_group(pv, 0)
                    av_group(pv, 1)
                    av_group(pv, 2)
                elif j == 1:
                    av_group(pv, 3)
                    av_group(pv, 4)
                    av_group(pv, 5)
                    rsb = a_sb.tile([128, 512], F32R, tag="rsb")
                    nc.vector.tensor_copy(rsb[:], pv["av"][:])
                    pv["rsb"] = rsb
                elif j == 2:
                    ex = a_xp.tile([128, 512], F32, tag="xp", name=f"ex{w}")
                    nc.tensor.matmul(
                        ex[:, 0:256], E128[:], pv["rsb"][:, 0:256],
                        start=True, stop=True,
                    )
                    nc.tensor.matmul(
                        ex[:, 256:512], E64[:], pv["rsb"][0:64, 256:512],
                        start=True, stop=True,
                    )
                    exc = a_sb.tile([128, 512], F32, tag="exc")
                    nc.vector.tensor_copy(exc[:], ex[:])
                    ex_sb = a_sb.tile([128, 512], F32, tag="exsb")
                    nc.vector.reciprocal_approx_fast(ex_sb[:], exc[:])
                    att = a_sb.tile([128, 512], BF16, tag="att")
                    nc.vector.tensor_tensor(
                        att[:], pv["av"][:], ex_sb[:], op=mybir.AluOpType.mult
                    )
                    pv["att"] = att
                elif j == 4:
                    wrl, wc = w // 16, w % 16
                    c0 = WS * wc
                    att = pv["att"]
                    pp = a_xp.tile([128, 512], F32, tag="xp", name=f"pp{w}")
                    for qc in range(2):
                        nc.tensor.matmul(
                            pp[:, 180 * qc : 180 * qc + 180],
                            att[:, 128 * qc : 128 * (qc + 1)],
                            WP0[:],
                            start=True,
                            stop=False,
                        )
                        nc.tensor.matmul(
                            pp[:, 180 * qc : 180 * qc + 180],
                            att[0:64, 256 + 128 * qc : 256 + 128 * (qc + 1)],
                            WP1[:],
                            start=False,
                            stop=True,
                        )
                    ot = a_sb.tile([128, 360], F32, tag="ot")
                    nc.vector.tensor_tensor(
                        ot[:], pp[:, 0:360], pv["xres"][:], op=mybir.AluOpType.add
                    )
                    for qc in range(2):
                        rq = WS * wrl + 8 * qc
                        nc.sync.dma_start(
                            out_pat[rq : rq + 8, c0 : c0 + WS, :],
                            ot[:, 180 * qc : 180 * qc + 180],
                        )

            vw = emit_vw(0)
            kws, qws = emit_kw(0)
            prev = None
            for w in range(NWIN):
                wrl, wc = w // 16, w % 16
                r0, c0 = WS * wrl, WS * wc
                xres = emit_xres(w)
                # prefetch next window's k/q tiles and v chunks FIRST so the
                # DVE copies land a full window before S^T(w+1) reads them
                if w + 1 < NWIN:
                    nvw = emit_vw(w + 1)
                    nkws, nqws = emit_kw(w + 1)

                es_list = []
                for j in range(5):
                    kc = CHUNK_KC[j]
                    st = a_st.tile([128, 1536], F32, tag="st", name=f"st{w}_{j}")
                    for h in range(NH):
                        ktile = kws[0] if h < 3 else kws[1]
                        qtile = qws[0] if h < 3 else qws[1]
                        hr = HROW[h]
                        nc.tensor.matmul(
                            st[0 : kc, STC[h] : STC[h] + 256],
                            ktile[hr : hr + 32, 120 * j : 120 * j + kc],
                            qtile[hr : hr + 32, :],
                            start=True,
                            stop=True,
                            tile_position=(hr, 0),
                        )
                    es = a_es.tile([128, 1536], BF16, tag="es", name=f"es{w}_{j}")
                    nc.scalar.activation(
                        es[0:kc, :], st[0:kc, :], mybir.ActivationFunctionType.Exp
                    )
                    es_list.append(es)
                    filler(prev, j)

                prev = {"w": w, "es": es_list, "vw": vw, "xres": xres}
                if w + 1 < NWIN:
                    vw = nvw
                    kws, qws = nkws, nqws

            for j in range(5):
                filler(prev, j)

    nc.compile()
    return nc


def _prep_host(inputs):
    x = np.ascontiguousarray(inputs["x"], dtype=np.float32)[0]  # [65536, 180]
    norm_w = np.asarray(inputs["norm_w"], np.float32)
    norm_b = np.asarray(inputs["norm_b"], np.float32)
    q_w = np.asarray(inputs["q_w"], np.float32)
    q_b = np.asarray(inputs["q_b"], np.float32)
    kv_w = np.asarray(inputs["kv_w"], np.float32)
    kv_b = np.asarray(inputs["kv_b"], np.float32)
    proj_w = np.asarray(inputs["proj_w"], np.float32)
    proj_b = np.asarray(inputs["proj_b"], np.float32)

    scale = HD ** -0.5
    Wq = norm_w[:, None] * q_w * scale
    bq = (norm_b @ q_w + q_b) * scale
    Wk = norm_w[:, None] * kv_w[:, :C]
    bk = norm_b @ kv_w[:, :C] + kv_b[:C]
    Wv = norm_w[:, None] * kv_w[:, C:]
    bv = norm_b @ kv_w[:, C:] + kv_b[C:]

    # wqk [181, 512]: 4 M-chunks (q h0-3 | q h4-5 | k h0-3 | k h4-5), 32-col head blocks
    wqk = np.zeros((181, 512), np.float32)
    for h in range(NH):
        mc = 0 if h < 4 else 1
        col = 128 * mc + 32 * (h % 4)
        wqk[:C, col : col + HD] = Wq[:, HD * h : HD * (h + 1)]
        wqk[C, col : col + HD] = bq[HD * h : HD * (h + 1)]
        colk = 256 + col
        wqk[:C, colk : colk + HD] = Wk[:, HD * h : HD * (h + 1)]
        wqk[C, colk : colk + HD] = bk[HD * h : HD * (h + 1)]

    # wv [181, 192]: 32-col head blocks; cols 30/31 of each block = ones (bias row)
    wv = np.zeros((181, 192), np.float32)
    for h in range(NH):
        col = 32 * h
        wv[:C, col : col + HD] = Wv[:, HD * h : HD * (h + 1)]
        wv[C, col : col + HD] = bv[HD * h : HD * (h + 1)]
        wv[C, col + 30] = 1.0
        wv[C, col + 31] = 1.0

    # wp [192, 180]: head-padded proj rows
    wp = np.zeros((192, C), np.float32)
    for h in range(NH):
        row = 32 * (h % 4) if h < 4 else 128 + 32 * (h - 4)
        wp[row : row + HD, :] = proj_w[HD * h : HD * (h + 1), :]

    ident = np.eye(128, dtype=bfnp)
    e128 = np.zeros((128, 128), np.float32)
    for j in range(4):
        e128[32 * j + 30, 32 * j : 32 * j + 32] = 1.0
    e64 = np.zeros((64, 128), np.float32)
    for j in range(2):
        e64[32 * j + 30, 32 * j : 32 * j + 32] = 1.0
        e64[32 * j + 30, 64 + 32 * j : 64 + 32 * j + 32] = 1.0

    # per-core slabs
    xg = x.reshape(H, W, C)
    xpad = np.zeros((H + 2 * PADW, CS, C), np.float32)
    xpad[PADW : PADW + H, PADW : PADW + W, :] = xg
    xres_full = x + proj_b  # residual (+proj bias folded in)

    in_maps = []
    for c in range(NCORES):
        slab = np.zeros((T, C), np.float32)
        slab[:TS_REAL] = xpad[32 * c : 32 * c + RS].reshape(TS_REAL, C)
        xr = np.ascontiguousarray(
            xres_full[8192 * c : 8192 * (c + 1)], dtype=np.float32
        )
        in_maps.append(
            {
                "xs": slab,
                "xr": xr,
                "wqk": wqk.astype(bfnp),
                "wv": wv.astype(bfnp),
                "wp": wp.astype(bfnp),
                "ident": ident,
                "e128": e128,
                "e64": e64,
                "ones": np.ones((1, T), bfnp),
            }
        )
    return in_maps


def kernel(**inputs):
    global _CACHED, LAST_RESULTS
    if _CACHED is None:
        _CACHED = _build_program()
    nc = _CACHED
    in_maps = _prep_host(inputs)
    res = run_bass_kernel_spmd(
        nc,
        in_maps,
        list(range(NCORES)),
        trace=False,
    )
    LAST_RESULTS = res
    out = np.empty((1, H * W, C), np.float32)
    for c in range(NCORES):
        out[0, 8192 * c : 8192 * (c + 1), :] = res.results[c]["out"]
    return out


# revision 41
# speedup vs baseline: 1.1961x; 1.1961x over previous
"""OCAB (overlapping cross-attention block) Trainium2 Bass kernel.

Full inputs in, full outputs out; internally shards the B*nW window axis
across 8 NeuronCores (each core owns 2 window-rows = 32 image rows, with a
4-row halo for the overlapping k/v windows).

v2 redesign vs the original baseline:
  - LayerNorm runs on DVE+ACT (bn_stats/bn_aggr, Sqrt with fused +eps bias,
    reciprocal, tensor_scalar) -- no GpSimd in the hot path.
  - V is projected into a token-major SBUF slab, then staged to a DRAM
    scratch once; each window's 5 key-chunks are gathered back with ONE
    DMA per chunk (column-major chunk layout makes the access pattern
    affine), replacing ~25 tiny SBUF-SBUF DMAs per window.
  - S^T PSUM packs all 6 heads in [128, 1536] (3 banks); one exp per chunk.
  - attn@V has no duplicate fill matmuls.
  - Windows are software-pipelined: the denominator-broadcast and
    projection matmuls of window w-1 are interleaved into window w's
    S^T stream so the PE never drains (keeps the HAM clock warm).
"""

import os
import sys
from contextlib import ExitStack

import numpy as np
import ml_dtypes

for _p in ("/opt/trn_rl_repo", "/root/.axon_site/_ro/trn_rl_repo"):
    if os.path.isdir(_p) and _p not in sys.path:
        sys.path.append(_p)

import concourse.bass as bass
import concourse.tile as tile
from concourse import bacc, mybir
from concourse.bass_utils import run_bass_kernel_spmd

BF16 = mybir.dt.bfloat16
F32 = mybir.dt.float32
F32R = mybir.dt.float32r
bfnp = ml_dtypes.bfloat16

# ---- problem constants (hardcoded per contract) ----
C = 180
NH = 6
HD = 30
WS = 16
OWS = 24
PADW = 4
H = W = 256
EPS = 1e-5
NCORES = 8

# ---- per-core slab geometry ----
RS = 40          # slab image rows (32 + 2*4 halo)
CS = 264         # slab image cols (256 + 2*4 zero pad)
TS_REAL = RS * CS          # 10560 real slab tokens
TCH = 84                   # token chunks of 128
T = TCH * 128              # 10752 padded slab tokens
NG = 21                    # 512-token groups (21*512 == 10752)
NWIN = 32                  # windows per core (2 window-rows x 16)
# column-major key chunks: chunk j covers patch cols [5j, 5j+ncol), all 24
# patch rows; within-chunk key index = 24*c_local + r.
CHUNK_NCOL = [5, 5, 5, 5, 4]
CHUNK_KC = [24 * n for n in CHUNK_NCOL]      # 120,120,120,120,96
# head -> st/es column; same-bank pairs (h0,h4)(h1,h5)(h2,h3) share a PE
# row group (A-tile rows h0/h1/h2 at 0/32/64, B-tile rows h4/h5/h3), so the
# 3-bank st can double-buffer without illegal concurrent same-bank writes.
STC = {0: 0, 4: 256, 1: 512, 5: 768, 2: 1024, 3: 1280}
HROW = {0: 0, 1: 32, 2: 64, 4: 0, 5: 32, 3: 64}

LAST_RESULTS = None
_CACHED = None


def _build_program():
    nc = bacc.Bacc("TRN2", target_bir_lowering=False)

    xs_d = nc.declare_dram_parameter("xs", [T, C], F32, isOutput=False)
    xr_d = nc.declare_dram_parameter("xr", [8192, C], F32, isOutput=False)
    wqk_d = nc.declare_dram_parameter("wqk", [181, 512], BF16, isOutput=False)
    wv_d = nc.declare_dram_parameter("wv", [181, 192], BF16, isOutput=False)
    wp_d = nc.declare_dram_parameter("wp", [192, C], BF16, isOutput=False)
    id_d = nc.declare_dram_parameter("ident", [128, 128], BF16, isOutput=False)
    e128_d = nc.declare_dram_parameter("e128", [128, 128], F32R, isOutput=False)
    e64_d = nc.declare_dram_parameter("e64", [64, 128], F32R, isOutput=False)
    ones_d = nc.declare_dram_parameter("ones", [1, T], BF16, isOutput=False)
    out_d = nc.declare_dram_parameter("out", [8192, C], F32, isOutput=True)

    with ExitStack() as ctx:
        tc = ctx.enter_context(tile.TileContext(nc))

        # ---- persistent: weights, q/k transposed slabs, V DRAM scratch ----
        wp_pool = ctx.enter_context(tc.tile_pool(name="wts", bufs=1))
        WQK0 = wp_pool.tile([128, 512], BF16, tag="wqk0")
        WQK1 = wp_pool.tile([53, 512], BF16, tag="wqk1")
        WV0 = wp_pool.tile([128, 192], BF16, tag="wv0")
        WV1 = wp_pool.tile([53, 192], BF16, tag="wv1")
        WP0 = wp_pool.tile([128, C], BF16, tag="wp0")
        WP1 = wp_pool.tile([64, C], BF16, tag="wp1")
        IDT = wp_pool.tile([128, 128], BF16, tag="id")
        E128 = wp_pool.tile([128, 128], F32R, tag="e128")
        E64 = wp_pool.tile([64, 128], F32R, tag="e64")
        EPSB = wp_pool.tile([128, 1], F32, tag="epsb")
        nc.vector.memset(EPSB[:], EPS)

        slab = ctx.enter_context(tc.tile_pool(name="slab", bufs=1))
        QS = [
            slab.tile([128, T], BF16, tag=f"qs{i}", name=f"qs{i}") for i in range(4)
        ]

        dram = ctx.enter_context(tc.tile_pool(name="dram", bufs=1, space="DRAM"))
        VD = dram.tile([T, 192], BF16, tag="vd")

        nc.sync.dma_start(WQK0[:], wqk_d[0:128, :])
        nc.sync.dma_start(WQK1[:], wqk_d[128:181, :])
        nc.sync.dma_start(WV0[:], wv_d[0:128, :])
        nc.sync.dma_start(WV1[:], wv_d[128:181, :])
        nc.sync.dma_start(WP0[:], wp_d[0:128, :])
        nc.sync.dma_start(WP1[:], wp_d[128:192, :])
        nc.sync.dma_start(IDT[:], id_d[:, :])
        nc.sync.dma_start(E128[:], e128_d[:, :])
        nc.sync.dma_start(E64[:], e64_d[:, :])

        # ================= phase 1+2: LN, transpose, projections ============
        with ExitStack() as pctx:
            slab2 = pctx.enter_context(tc.tile_pool(name="slab2", bufs=1))
            XT0 = slab2.tile([128, T], BF16, tag="xt0")   # xn^T channels 0-127
            XT1 = slab2.tile([53, T], BF16, tag="xt1")    # ch 128-179 + ones row
            VS = slab2.tile([128, TCH * 192], BF16, tag="vs")
            nc.sync.dma_start(XT1[52:53, :], ones_d[:, :])

            p_x = pctx.enter_context(tc.tile_pool(name="p_x", bufs=8))
            p_sm = pctx.enter_context(tc.tile_pool(name="p_sm", bufs=10))
            p_ps = pctx.enter_context(tc.tile_pool(name="p_ps", bufs=2, space="PSUM"))

            def emit_proj(g):
                """projections for token group g (q skips halo groups); the
                LN loop emits group g-1's projections so the PE always has
                dense N=512 streams to chew on between transposes."""
                for mc in range(4):
                    if mc < 2 and not (2 <= g <= 18):
                        continue
                    qp = p_ps.tile([128, 512], F32, tag="mm")
                    nc.tensor.matmul(
                        qp[:],
                        WQK0[:, 128 * mc : 128 * (mc + 1)],
                        XT0[:, 512 * g : 512 * (g + 1)],
                        start=True,
                        stop=False,
                    )
                    nc.tensor.matmul(
                        qp[:],
                        WQK1[:, 128 * mc : 128 * (mc + 1)],
                        XT1[:, 512 * g : 512 * (g + 1)],
                        start=False,
                        stop=True,
                    )
                    if (g + mc) % 2 == 0:
                        nc.vector.tensor_copy(
                            QS[mc][:, 512 * g : 512 * (g + 1)], qp[:]
                        )
                    else:
                        nc.scalar.copy(QS[mc][:, 512 * g : 512 * (g + 1)], qp[:])
                for pair in (2 * g, 2 * g + 1):
                    vp = p_ps.tile([128, 384], F32, tag="vmm")
                    for j in range(2):
                        tch = 2 * pair + j
                        nc.tensor.matmul(
                            vp[:, 192 * j : 192 * (j + 1)],
                            XT0[:, 128 * tch : 128 * (tch + 1)],
                            WV0[:],
                            start=True,
                            stop=False,
                        )
                        nc.tensor.matmul(
                            vp[:, 192 * j : 192 * (j + 1)],
                            XT1[:, 128 * tch : 128 * (tch + 1)],
                            WV1[:],
                            start=False,
                            stop=True,
                        )
                    if pair % 2 == 0:
                        nc.vector.tensor_copy(
                            VS[:, 384 * pair : 384 * (pair + 1)], vp[:]
                        )
                    else:
                        nc.scalar.copy(VS[:, 384 * pair : 384 * (pair + 1)], vp[:])
                # stage this group's V to DRAM (token-major) while later
                # groups are still projecting
                nc.sync.dma_start(
                    VD[512 * g : 512 * (g + 1), :].rearrange(
                        "(b p) v -> p b v", p=128
                    ),
                    VS[:, 768 * g : 768 * (g + 1)].rearrange(
                        "p (b v) -> p b v", v=192
                    ),
                )

            for g in range(TCH // 4):          # 21 groups of 4 token chunks
                tp = p_ps.tile([128, 512], BF16, tag="tp")
                tp2 = p_ps.tile([52, 512], BF16, tag="tp2")
                for j2 in range(2):
                    xt2 = p_x.tile([128, 360], F32, tag="x")
                    t2 = 2 * g + j2
                    nc.sync.dma_start(
                        xt2[:].rearrange("p (b c) -> p b c", c=C),
                        xs_d[256 * t2 : 256 * (t2 + 1), :].rearrange(
                            "(b p) c -> p b c", p=128
                        ),
                    )
                    for jh in range(2):
                        j = 2 * j2 + jh
                        xt = xt2[:, C * jh : C * (jh + 1)]
                        stats = p_sm.tile([128, 6], F32, tag="st")
                        aggr = p_sm.tile([128, 2], F32, tag="ag")
                        nc.vector.bn_stats(stats[:], xt)
                        nc.vector.bn_aggr(aggr[:], stats[:])
                        sd = p_sm.tile([128, 1], F32, tag="sd")
                        nc.scalar.activation(
                            sd[:], aggr[:, 1:2], mybir.ActivationFunctionType.Sqrt,
                            bias=EPSB[:, 0:1],
                        )
                        rstd = p_sm.tile([128, 1], F32, tag="rstd")
                        nc.vector.reciprocal(rstd[:], sd[:])
                        xn = p_x.tile([128, C], BF16, tag="xn")
                        nc.vector.tensor_scalar(
                            xn[:],
                            xt,
                            aggr[:, 0:1],
                            rstd[:],
                            op0=mybir.AluOpType.subtract,
                            op1=mybir.AluOpType.mult,
                        )
                        nc.tensor.transpose(
                            tp[:, 128 * j : 128 * (j + 1)], xn[:, 0:128], IDT[:]
                        )
                        nc.tensor.transpose(
                            tp2[:, 128 * j : 128 * (j + 1)], xn[:, 128:180], IDT[:]
                        )
                if g % 2 == 0:
                    nc.scalar.copy(XT0[:, 512 * g : 512 * (g + 1)], tp[:])
                    nc.vector.tensor_copy(XT1[0:52, 512 * g : 512 * (g + 1)], tp2[:])
                else:
                    nc.vector.tensor_copy(XT0[:, 512 * g : 512 * (g + 1)], tp[:])
                    nc.scalar.copy(XT1[0:52, 512 * g : 512 * (g + 1)], tp2[:])
                if g > 0:
                    emit_proj(g - 1)
            emit_proj(TCH // 4 - 1)

        # ================= phase 3: windowed attention =======================
        with ExitStack() as actx:
            a_st = actx.enter_context(tc.tile_pool(name="a_st", bufs=2, space="PSUM"))
            a_av = actx.enter_context(tc.tile_pool(name="a_av", bufs=1, space="PSUM"))
            a_xp = actx.enter_context(tc.tile_pool(name="a_xp", bufs=1, space="PSUM"))
            a_es = actx.enter_context(tc.tile_pool(name="a_es", bufs=11))
            a_vw = actx.enter_context(tc.tile_pool(name="a_vw", bufs=16))
            a_kw = actx.enter_context(tc.tile_pool(name="a_kw", bufs=2))
            a_sb = actx.enter_context(tc.tile_pool(name="a_sb", bufs=2))

            # r-major per-window views of the q slabs (for the moving q operand)
            qs_pat = [
                QS[i][:, 0:TS_REAL].rearrange("p (r c) -> p r c", c=CS)
                for i in range(4)
            ]
            # c-major views of the k slabs (for the kw materialization)
            qs_cpat = [
                QS[i][:, 0:TS_REAL].rearrange("p (r c) -> p c r", c=CS)
                for i in range(4)
            ]
            # c-major view of the V DRAM scratch
            vd_c = VD[0:TS_REAL, :].rearrange("(r c) v -> c r v", c=CS)

            xr_pat = xr_d[:, :].rearrange("(r c) d -> r c d", c=W)
            out_pat = out_d[:, :].rearrange("(r c) d -> r c d", c=W)

            def emit_vw(w):
                """one DMA per key chunk: [ncol, 24, 192] dram -> [kc, 192]"""
                wrl, wc = w // 16, w % 16
                r0, c0 = WS * wrl, WS * wc
                tiles = []
                for j in range(5):
                    ncol, kc = CHUNK_NCOL[j], CHUNK_KC[j]
                    vw = a_vw.tile([128, 192], BF16, tag="vw", name=f"vw{w}_{j}")
                    nc.sync.dma_start(
                        vw[0:kc, :],
                        vd_c[c0 + 5 * j : c0 + 5 * j + ncol, r0 : r0 + 24, :],
                    )
                    tiles.append(vw)
                return tiles

            def emit_kw(w):
                """k^T/q window tiles packed as head triples: A = (h0,h1,h2)
                at rows 0/32/64, B = (h4,h5,h3) at rows 0/32/64 (c-major key
                order, col = 24c+r).  Copies stay wide and base-partition
                legal: [0:96], [0:64], [64:96]."""
                wrl, wc = w // 16, w % 16
                r0, c0 = WS * wrl, WS * wc
                kwA = a_kw.tile([96, 576], BF16, tag="kwA", name=f"kwA_{w}")
                kwB = a_kw.tile([96, 576], BF16, tag="kwB", name=f"kwB_{w}")
                qwA = a_kw.tile([96, 256], BF16, tag="qwA", name=f"qwA_{w}")
                qwB = a_kw.tile([96, 256], BF16, tag="qwB", name=f"qwB_{w}")
                kA = kwA[:].rearrange("p (c r) -> p c r", r=OWS)
                kB = kwB[:].rearrange("p (c r) -> p c r", r=OWS)
                ksl = (slice(c0, c0 + OWS), slice(r0, r0 + OWS))
                nc.vector.tensor_copy(kA, qs_cpat[2][0:96, ksl[0], ksl[1]])
                nc.vector.tensor_copy(kB[0:64], qs_cpat[3][0:64, ksl[0], ksl[1]])
                nc.vector.tensor_copy(kB[64:96], qs_cpat[2][96:128, ksl[0], ksl[1]])
                qA = qwA[:].rearrange("p (a b) -> p a b", b=WS)
                qB = qwB[:].rearrange("p (a b) -> p a b", b=WS)
                qsl = (
                    slice(PADW + WS * wrl, PADW + WS * wrl + WS),
                    slice(PADW + c0, PADW + c0 + WS),
                )
                nc.vector.tensor_copy(qA, qs_pat[0][0:96, qsl[0], qsl[1]])
                nc.vector.tensor_copy(qB[0:64], qs_pat[1][0:64, qsl[0], qsl[1]])
                nc.vector.tensor_copy(qB[64:96], qs_pat[0][96:128, qsl[0], qsl[1]])
                return (kwA, kwB), (qwA, qwB)

            def emit_xres(w):
                wrl, wc = w // 16, w % 16
                c0 = WS * wc
                xres = a_sb.tile([128, 360], F32, tag="xres", name=f"xres{w}", bufs=3)
                for qc in range(2):
                    rq = WS * wrl + 8 * qc
                    nc.gpsimd.dma_start(
                        xres[:, 180 * qc : 180 * qc + 180],
                        xr_pat[rq : rq + 8, c0 : c0 + WS, :],
                    )
                return xres

            def av_group(pv, h):
                """one contiguous 5-matmul PSUM accumulation group for head h
                of the previous window (groups must never interleave)."""
                colp, colf = 32 * (h % 4), 256 * (h // 4)
                for j in range(5):
                    kc = CHUNK_KC[j]
                    nc.tensor.matmul(
                        pv["av"][colp : colp + 32, colf : colf + 256],
                        pv["vw"][j][0:kc, 32 * h : 32 * h + 32],
                        pv["es"][j][0:kc, STC[h] : STC[h] + 256],
                        start=(j == 0),
                        stop=(j == 4),
                        tile_position=(0, colp),
                    )

            def filler(pv, j):
                """previous-window work dropped into round j's exp shadow"""
                if pv is None:
                    return
                w = pv["w"]
                if j == 0:
                    pv["av"] = a_av.tile([128, 512], F32, tag="av", name=f"av{w}")
                    av_group(pv, 0)
                    av_group(pv, 1)
                    av_group(pv, 2)
                elif j == 1:
                    av_group(pv, 3)
                    av_group(pv, 4)
                    av_group(pv, 5)
                    rsb = a_sb.tile([128, 512], F32R, tag="rsb")
                    nc.vector.tensor_copy(rsb[:], pv["av"][:])
                    pv["rsb"] = rsb
                elif j == 2:
                    ex = a_xp.tile([128, 512], F32, tag="xp", name=f"ex{w}")
                    nc.tensor.matmul(
                        ex[:, 0:256], E128[:], pv["rsb"][:, 0:256],
                        start=True, stop=True,
                    )
                    nc.tensor.matmul(
                        ex[:, 256:512], E64[:], pv["rsb"][0:64, 256:512],
                        start=True, stop=True,
                    )
                    exc = a_sb.tile([128, 512], F32, tag="exc")
                    nc.vector.tensor_copy(exc[:], ex[:])
                    ex_sb = a_sb.tile([128, 512], F32, tag="exsb")
                    nc.vector.reciprocal_approx_fast(ex_sb[:], exc[:])
                    att = a_sb.tile([128, 512], BF16, tag="att")
                    nc.vector.tensor_tensor(
                        att[:], pv["av"][:], ex_sb[:], op=mybir.AluOpType.mult
                    )
                    pv["att"] = att
                elif j == 3:
                    wrl, wc = w // 16, w % 16
                    c0 = WS * wc
                    att = pv["att"]
                    pp = a_xp.tile([128, 512], F32, tag="xp", name=f"pp{w}")
                    for qc in range(2):
                        nc.tensor.matmul(
                            pp[:, 180 * qc : 180 * qc + 180],
                            att[:, 128 * qc : 128 * (qc + 1)],
                            WP0[:],
                            start=True,
                            stop=False,
                        )
                        nc.tensor.matmul(
                            pp[:, 180 * qc : 180 * qc + 180],
                            att[0:64, 256 + 128 * qc : 256 + 128 * (qc + 1)],
                            WP1[:],
                            start=False,
                            stop=True,
                        )
                    ot = a_sb.tile([128, 360], F32, tag="ot")
                    nc.vector.tensor_tensor(
                        ot[:], pp[:, 0:360], pv["xres"][:], op=mybir.AluOpType.add
                    )
                    for qc in range(2):
                        rq = WS * wrl + 8 * qc
                        nc.sync.dma_start(
                            out_pat[rq : rq + 8, c0 : c0 + WS, :],
                            ot[:, 180 * qc : 180 * qc + 180],
                        )

            vw = emit_vw(0)
            kws, qws = emit_kw(0)
            prev = None
            for w in range(NWIN):
                wrl, wc = w // 16, w % 16
                r0, c0 = WS * wrl, WS * wc
                xres = emit_xres(w)
                # prefetch next window's k/q tiles and v chunks FIRST so the
                # DVE copies land a full window before S^T(w+1) reads them
                if w + 1 < NWIN:
                    nvw = emit_vw(w + 1)
                    nkws, nqws = emit_kw(w + 1)

                es_list = []
                for j in range(5):
                    kc = CHUNK_KC[j]
                    st = a_st.tile([128, 1536], F32, tag="st", name=f"st{w}_{j}")
                    for h in range(NH):
                        ktile = kws[0] if h < 3 else kws[1]
                        qtile = qws[0] if h < 3 else qws[1]
                        hr = HROW[h]
                        nc.tensor.matmul(
                            st[0 : kc, STC[h] : STC[h] + 256],
                            ktile[hr : hr + 32, 120 * j : 120 * j + kc],
                            qtile[hr : hr + 32, :],
                            start=True,
                            stop=True,
                            tile_position=(hr, 0),
                        )
                    es = a_es.tile([128, 1536], BF16, tag="es", name=f"es{w}_{j}")
                    nc.scalar.activation(
                        es[0:kc, :], st[0:kc, :], mybir.ActivationFunctionType.Exp
                    )
                    es_list.append(es)
                    filler(prev, j)

                prev = {"w": w, "es": es_list, "vw": vw, "xres": xres}
                if w + 1 < NWIN:
                    vw = nvw
                    kws, qws = nkws, nqws

            for j in range(5):
                filler(prev, j)

    nc.compile()
    return nc


def _prep_host(inputs):
    x = np.ascontiguousarray(inputs["x"], dtype=np.float32)[0]  # [65536, 180]
    norm_w = np.asarray(inputs["norm_w"], np.float32)
    norm_b = np.asarray(inputs["norm_b"], np.float32)
    q_w = np.asarray(inputs["q_w"], np.float32)
    q_b = np.asarray(inputs["q_b"], np.float32)
    kv_w = np.asarray(inputs["kv_w"], np.float32)
    kv_b = np.asarray(inputs["kv_b"], np.float32)
    proj_w = np.asarray(inputs["proj_w"], np.float32)
    proj_b = np.asarray(inputs["proj_b"], np.float32)

    scale = HD ** -0.5
    Wq = norm_w[:, None] * q_w * scale
    bq = (norm_b @ q_w + q_b) * scale
    Wk = norm_w[:, None] * kv_w[:, :C]
    bk = norm_b @ kv_w[:, :C] + kv_b[:C]
    Wv = norm_w[:, None] * kv_w[:, C:]
    bv = norm_b @ kv_w[:, C:] + kv_b[C:]

    # wqk [181, 512]: 4 M-chunks (q h0-3 | q h4-5 | k h0-3 | k h4-5), 32-col head blocks
    wqk = np.zeros((181, 512), np.float32)
    for h in range(NH):
        mc = 0 if h < 4 else 1
        col = 128 * mc + 32 * (h % 4)
        wqk[:C, col : col + HD] = Wq[:, HD * h : HD * (h + 1)]
        wqk[C, col : col + HD] = bq[HD * h : HD * (h + 1)]
        colk = 256 + col
        wqk[:C, colk : colk + HD] = Wk[:, HD * h : HD * (h + 1)]
        wqk[C, colk : colk + HD] = bk[HD * h : HD * (h + 1)]

    # wv [181, 192]: 32-col head blocks; cols 30/31 of each block = ones (bias row)
    wv = np.zeros((181, 192), np.float32)
    for h in range(NH):
        col = 32 * h
        wv[:C, col : col + HD] = Wv[:, HD * h : HD * (h + 1)]
        wv[C, col : col + HD] = bv[HD * h : HD * (h + 1)]
        wv[C, col + 30] = 1.0
        wv[C, col + 31] = 1.0

    # wp [192, 180]: head-padded proj rows
    wp = np.zeros((192, C), np.float32)
    for h in range(NH):
        row = 32 * (h % 4) if h < 4 else 128 + 32 * (h - 4)
        wp[row : row + HD, :] = proj_w[HD * h : HD * (h + 1), :]

    ident = np.eye(128, dtype=bfnp)
    e128 = np.zeros((128, 128), np.float32)
    for j in range(4):
        e128[32 * j + 30, 32 * j : 32 * j + 32] = 1.0
    e64 = np.zeros((64, 128), np.float32)
    for j in range(2):
        e64[32 * j + 30, 32 * j : 32 * j + 32] = 1.0
        e64[32 * j + 30, 64 + 32 * j : 64 + 32 * j + 32] = 1.0

    # per-core slabs
    xg = x.reshape(H, W, C)
    xpad = np.zeros((H + 2 * PADW, CS, C), np.float32)
    xpad[PADW : PADW + H, PADW : PADW + W, :] = xg
    xres_full = x + proj_b  # residual (+proj bias folded in)

    in_maps = []
    for c in range(NCORES):
        slab = np.zeros((T, C), np.float32)
        slab[:TS_REAL] = xpad[32 * c : 32 * c + RS].reshape(TS_REAL, C)
        xr = np.ascontiguousarray(
            xres_full[8192 * c : 8192 * (c + 1)], dtype=np.float32
        )
        in_maps.append(
            {
                "xs": slab,
                "xr": xr,
                "wqk": wqk.astype(bfnp),
                "wv": wv.astype(bfnp),
                "wp": wp.astype(bfnp),
                "ident": ident,
                "e128": e128,
                "e64": e64,
                "ones": np.ones((1, T), bfnp),
            }
        )
    return in_maps


def kernel(**inputs):
    global _CACHED, LAST_RESULTS
    if _CACHED is None:
        _CACHED = _build_program()
    nc = _CACHED
    in_maps = _prep_host(inputs)
    res = run_bass_kernel_spmd(
        nc,
        in_maps,
        list(range(NCORES)),
        trace=False,
    )
    LAST_RESULTS = res
    out = np.empty((1, H * W, C), np.float32)
    for c in range(NCORES):
        out[0, 8192 * c : 8192 * (c + 1), :] = res.results[c]["out"]
    return out


# revision 43
# speedup vs baseline: 1.1967x; 1.0005x over previous
"""OCAB (overlapping cross-attention block) Trainium2 Bass kernel.

Full inputs in, full outputs out; internally shards the B*nW window axis
across 8 NeuronCores (each core owns 2 window-rows = 32 image rows, with a
4-row halo for the overlapping k/v windows).

v2 redesign vs the original baseline:
  - LayerNorm runs on DVE+ACT (bn_stats/bn_aggr, Sqrt with fused +eps bias,
    reciprocal, tensor_scalar) -- no GpSimd in the hot path.
  - V is projected into a token-major SBUF slab, then staged to a DRAM
    scratch once; each window's 5 key-chunks are gathered back with ONE
    DMA per chunk (column-major chunk layout makes the access pattern
    affine), replacing ~25 tiny SBUF-SBUF DMAs per window.
  - S^T PSUM packs all 6 heads in [128, 1536] (3 banks); one exp per chunk.
  - attn@V has no duplicate fill matmuls.
  - Windows are software-pipelined: the denominator-broadcast and
    projection matmuls of window w-1 are interleaved into window w's
    S^T stream so the PE never drains (keeps the HAM clock warm).
"""

import os
import sys
from contextlib import ExitStack

import numpy as np
import ml_dtypes

for _p in ("/opt/trn_rl_repo", "/root/.axon_site/_ro/trn_rl_repo"):
    if os.path.isdir(_p) and _p not in sys.path:
        sys.path.append(_p)

import concourse.bass as bass
import concourse.tile as tile
from concourse import bacc, mybir
from concourse.bass_utils import run_bass_kernel_spmd

BF16 = mybir.dt.bfloat16
F32 = mybir.dt.float32
F32R = mybir.dt.float32r
bfnp = ml_dtypes.bfloat16

# ---- problem constants (hardcoded per contract) ----
C = 180
NH = 6
HD = 30
WS = 16
OWS = 24
PADW = 4
H = W = 256
EPS = 1e-5
NCORES = 8

# ---- per-core slab geometry ----
RS = 40          # slab image rows (32 + 2*4 halo)
CS = 264         # slab image cols (256 + 2*4 zero pad)
TS_REAL = RS * CS          # 10560 real slab tokens
TCH = 84                   # token chunks of 128
T = TCH * 128              # 10752 padded slab tokens
NG = 21                    # 512-token groups (21*512 == 10752)
NWIN = 32                  # windows per core (2 window-rows x 16)
# column-major key chunks: chunk j covers patch cols [5j, 5j+ncol), all 24
# patch rows; within-chunk key index = 24*c_local + r.
CHUNK_NCOL = [5, 5, 5, 5, 4]
CHUNK_KC = [24 * n for n in CHUNK_NCOL]      # 120,120,120,120,96
# head -> st/es column; same-bank pairs (h0,h4)(h1,h5)(h2,h3) share a PE
# row group (A-tile rows h0/h1/h2 at 0/32/64, B-tile rows h4/h5/h3), so the
# 3-bank st can double-buffer without illegal concurrent same-bank writes.
STC = {0: 0, 4: 256, 1: 512, 5: 768, 2: 1024, 3: 1280}
HROW = {0: 0, 1: 32, 2: 64, 4: 0, 5: 32, 3: 64}

LAST_RESULTS = None
_CACHED = None


def _build_program():
    nc = bacc.Bacc("TRN2", target_bir_lowering=False)

    xs_d = nc.declare_dram_parameter("xs", [T, C], F32, isOutput=False)
    xr_d = nc.declare_dram_parameter("xr", [8192, C], F32, isOutput=False)
    wqk_d = nc.declare_dram_parameter("wqk", [181, 512], BF16, isOutput=False)
    wv_d = nc.declare_dram_parameter("wv", [181, 192], BF16, isOutput=False)
    wp_d = nc.declare_dram_parameter("wp", [192, C], BF16, isOutput=False)
    id_d = nc.declare_dram_parameter("ident", [128, 128], BF16, isOutput=False)
    e128_d = nc.declare_dram_parameter("e128", [128, 128], F32R, isOutput=False)
    e64_d = nc.declare_dram_parameter("e64", [64, 128], F32R, isOutput=False)
    ones_d = nc.declare_dram_parameter("ones", [1, T], BF16, isOutput=False)
    out_d = nc.declare_dram_parameter("out", [8192, C], F32, isOutput=True)

    with ExitStack() as ctx:
        tc = ctx.enter_context(tile.TileContext(nc))

        # ---- persistent: weights, q/k transposed slabs, V DRAM scratch ----
        wp_pool = ctx.enter_context(tc.tile_pool(name="wts", bufs=1))
        WQK0 = wp_pool.tile([128, 512], BF16, tag="wqk0")
        WQK1 = wp_pool.tile([53, 512], BF16, tag="wqk1")
        WV0 = wp_pool.tile([128, 192], BF16, tag="wv0")
        WV1 = wp_pool.tile([53, 192], BF16, tag="wv1")
        WP0 = wp_pool.tile([128, C], BF16, tag="wp0")
        WP1 = wp_pool.tile([64, C], BF16, tag="wp1")
        IDT = wp_pool.tile([128, 128], BF16, tag="id")
        E128 = wp_pool.tile([128, 128], F32R, tag="e128")
        E64 = wp_pool.tile([64, 128], F32R, tag="e64")
        EPSB = wp_pool.tile([128, 1], F32, tag="epsb")
        nc.vector.memset(EPSB[:], EPS)

        slab = ctx.enter_context(tc.tile_pool(name="slab", bufs=1))
        QS = [
            slab.tile([128, T], BF16, tag=f"qs{i}", name=f"qs{i}") for i in range(4)
        ]

        dram = ctx.enter_context(tc.tile_pool(name="dram", bufs=1, space="DRAM"))
        VD = dram.tile([T, 192], BF16, tag="vd")

        nc.sync.dma_start(WQK0[:], wqk_d[0:128, :])
        nc.sync.dma_start(WQK1[:], wqk_d[128:181, :])
        nc.sync.dma_start(WV0[:], wv_d[0:128, :])
        nc.sync.dma_start(WV1[:], wv_d[128:181, :])
        nc.sync.dma_start(WP0[:], wp_d[0:128, :])
        nc.sync.dma_start(WP1[:], wp_d[128:192, :])
        nc.sync.dma_start(IDT[:], id_d[:, :])
        nc.sync.dma_start(E128[:], e128_d[:, :])
        nc.sync.dma_start(E64[:], e64_d[:, :])

        # ================= phase 1+2: LN, transpose, projections ============
        with ExitStack() as pctx:
            slab2 = pctx.enter_context(tc.tile_pool(name="slab2", bufs=1))
            XT0 = slab2.tile([128, T], BF16, tag="xt0")   # xn^T channels 0-127
            XT1 = slab2.tile([53, T], BF16, tag="xt1")    # ch 128-179 + ones row
            VS = slab2.tile([128, TCH * 192], BF16, tag="vs")
            nc.sync.dma_start(XT1[52:53, :], ones_d[:, :])

            p_x = pctx.enter_context(tc.tile_pool(name="p_x", bufs=8))
            p_sm = pctx.enter_context(tc.tile_pool(name="p_sm", bufs=10))
            p_ps = pctx.enter_context(tc.tile_pool(name="p_ps", bufs=2, space="PSUM"))

            def emit_proj(g):
                """projections for token group g (q skips halo groups); the
                LN loop emits group g-1's projections so the PE always has
                dense N=512 streams to chew on between transposes."""
                for mc in range(4):
                    if mc < 2 and not (2 <= g <= 18):
                        continue
                    qp = p_ps.tile([128, 512], F32, tag="mm")
                    nc.tensor.matmul(
                        qp[:],
                        WQK0[:, 128 * mc : 128 * (mc + 1)],
                        XT0[:, 512 * g : 512 * (g + 1)],
                        start=True,
                        stop=False,
                    )
                    nc.tensor.matmul(
                        qp[:],
                        WQK1[:, 128 * mc : 128 * (mc + 1)],
                        XT1[:, 512 * g : 512 * (g + 1)],
                        start=False,
                        stop=True,
                    )
                    if (g + mc) % 3 == 0:
                        nc.vector.tensor_copy(
                            QS[mc][:, 512 * g : 512 * (g + 1)], qp[:]
                        )
                    else:
                        nc.scalar.copy(QS[mc][:, 512 * g : 512 * (g + 1)], qp[:])
                for pair in (2 * g, 2 * g + 1):
                    vp = p_ps.tile([128, 384], F32, tag="vmm")
                    for j in range(2):
                        tch = 2 * pair + j
                        nc.tensor.matmul(
                            vp[:, 192 * j : 192 * (j + 1)],
                            XT0[:, 128 * tch : 128 * (tch + 1)],
                            WV0[:],
                            start=True,
                            stop=False,
                        )
                        nc.tensor.matmul(
                            vp[:, 192 * j : 192 * (j + 1)],
                            XT1[:, 128 * tch : 128 * (tch + 1)],
                            WV1[:],
                            start=False,
                            stop=True,
                        )
                    if pair % 3 == 0:
                        nc.vector.tensor_copy(
                            VS[:, 384 * pair : 384 * (pair + 1)], vp[:]
                        )
                    else:
                        nc.scalar.copy(VS[:, 384 * pair : 384 * (pair + 1)], vp[:])
                # stage this group's V to DRAM (token-major) while later
                # groups are still projecting
                nc.sync.dma_start(
                    VD[512 * g : 512 * (g + 1), :].rearrange(
                        "(b p) v -> p b v", p=128
                    ),
                    VS[:, 768 * g : 768 * (g + 1)].rearrange(
                        "p (b v) -> p b v", v=192
                    ),
                )

            for g in range(TCH // 4):          # 21 groups of 4 token chunks
                tp = p_ps.tile([128, 512], BF16, tag="tp")
                tp2 = p_ps.tile([52, 512], BF16, tag="tp2")
                for j2 in range(2):
                    xt2 = p_x.tile([128, 360], F32, tag="x")
                    t2 = 2 * g + j2
                    nc.sync.dma_start(
                        xt2[:].rearrange("p (b c) -> p b c", c=C),
                        xs_d[256 * t2 : 256 * (t2 + 1), :].rearrange(
                            "(b p) c -> p b c", p=128
                        ),
                    )
                    for jh in range(2):
                        j = 2 * j2 + jh
                        xt = xt2[:, C * jh : C * (jh + 1)]
                        stats = p_sm.tile([128, 6], F32, tag="st")
                        aggr = p_sm.tile([128, 2], F32, tag="ag")
                        nc.vector.bn_stats(stats[:], xt)
                        nc.vector.bn_aggr(aggr[:], stats[:])
                        sd = p_sm.tile([128, 1], F32, tag="sd")
                        nc.scalar.activation(
                            sd[:], aggr[:, 1:2], mybir.ActivationFunctionType.Sqrt,
                            bias=EPSB[:, 0:1],
                        )
                        rstd = p_sm.tile([128, 1], F32, tag="rstd")
                        nc.vector.reciprocal(rstd[:], sd[:])
                        xn = p_x.tile([128, C], BF16, tag="xn")
                        nc.vector.tensor_scalar(
                            xn[:],
                            xt,
                            aggr[:, 0:1],
                            rstd[:],
                            op0=mybir.AluOpType.subtract,
                            op1=mybir.AluOpType.mult,
                        )
                        nc.tensor.transpose(
                            tp[:, 128 * j : 128 * (j + 1)], xn[:, 0:128], IDT[:]
                        )
                        nc.tensor.transpose(
                            tp2[:, 128 * j : 128 * (j + 1)], xn[:, 128:180], IDT[:]
                        )
                if g % 2 == 0:
                    nc.scalar.copy(XT0[:, 512 * g : 512 * (g + 1)], tp[:])
                    nc.vector.tensor_copy(XT1[0:52, 512 * g : 512 * (g + 1)], tp2[:])
                else:
                    nc.vector.tensor_copy(XT0[:, 512 * g : 512 * (g + 1)], tp[:])
                    nc.scalar.copy(XT1[0:52, 512 * g : 512 * (g + 1)], tp2[:])
                if g > 0:
                    emit_proj(g - 1)
            emit_proj(TCH // 4 - 1)

        # ================= phase 3: windowed attention =======================
        with ExitStack() as actx:
            a_st = actx.enter_context(tc.tile_pool(name="a_st", bufs=2, space="PSUM"))
            a_av = actx.enter_context(tc.tile_pool(name="a_av", bufs=1, space="PSUM"))
            a_xp = actx.enter_context(tc.tile_pool(name="a_xp", bufs=1, space="PSUM"))
            a_es = actx.enter_context(tc.tile_pool(name="a_es", bufs=11))
            a_vw = actx.enter_context(tc.tile_pool(name="a_vw", bufs=16))
            a_kw = actx.enter_context(tc.tile_pool(name="a_kw", bufs=2))
            a_sb = actx.enter_context(tc.tile_pool(name="a_sb", bufs=2))

            # r-major per-window views of the q slabs (for the moving q operand)
            qs_pat = [
                QS[i][:, 0:TS_REAL].rearrange("p (r c) -> p r c", c=CS)
                for i in range(4)
            ]
            # c-major views of the k slabs (for the kw materialization)
            qs_cpat = [
                QS[i][:, 0:TS_REAL].rearrange("p (r c) -> p c r", c=CS)
                for i in range(4)
            ]
            # c-major view of the V DRAM scratch
            vd_c = VD[0:TS_REAL, :].rearrange("(r c) v -> c r v", c=CS)

            xr_pat = xr_d[:, :].rearrange("(r c) d -> r c d", c=W)
            out_pat = out_d[:, :].rearrange("(r c) d -> r c d", c=W)

            def emit_vw(w):
                """one DMA per key chunk: [ncol, 24, 192] dram -> [kc, 192]"""
                wrl, wc = w // 16, w % 16
                r0, c0 = WS * wrl, WS * wc
                tiles = []
                for j in range(5):
                    ncol, kc = CHUNK_NCOL[j], CHUNK_KC[j]
                    vw = a_vw.tile([128, 192], BF16, tag="vw", name=f"vw{w}_{j}")
                    nc.sync.dma_start(
                        vw[0:kc, :],
                        vd_c[c0 + 5 * j : c0 + 5 * j + ncol, r0 : r0 + 24, :],
                    )
                    tiles.append(vw)
                return tiles

            def emit_kw(w):
                """k^T/q window tiles packed as head triples: A = (h0,h1,h2)
                at rows 0/32/64, B = (h4,h5,h3) at rows 0/32/64 (c-major key
                order, col = 24c+r).  Copies stay wide and base-partition
                legal: [0:96], [0:64], [64:96]."""
                wrl, wc = w // 16, w % 16
                r0, c0 = WS * wrl, WS * wc
                kwA = a_kw.tile([96, 576], BF16, tag="kwA", name=f"kwA_{w}")
                kwB = a_kw.tile([96, 576], BF16, tag="kwB", name=f"kwB_{w}")
                qwA = a_kw.tile([96, 256], BF16, tag="qwA", name=f"qwA_{w}")
                qwB = a_kw.tile([96, 256], BF16, tag="qwB", name=f"qwB_{w}")
                kA = kwA[:].rearrange("p (c r) -> p c r", r=OWS)
                kB = kwB[:].rearrange("p (c r) -> p c r", r=OWS)
                ksl = (slice(c0, c0 + OWS), slice(r0, r0 + OWS))
                nc.vector.tensor_copy(kA, qs_cpat[2][0:96, ksl[0], ksl[1]])
                nc.vector.tensor_copy(kB[0:64], qs_cpat[3][0:64, ksl[0], ksl[1]])
                nc.vector.tensor_copy(kB[64:96], qs_cpat[2][96:128, ksl[0], ksl[1]])
                qA = qwA[:].rearrange("p (a b) -> p a b", b=WS)
                qB = qwB[:].rearrange("p (a b) -> p a b", b=WS)
                qsl = (
                    slice(PADW + WS * wrl, PADW + WS * wrl + WS),
                    slice(PADW + c0, PADW + c0 + WS),
                )
                nc.vector.tensor_copy(qA, qs_pat[0][0:96, qsl[0], qsl[1]])
                nc.vector.tensor_copy(qB[0:64], qs_pat[1][0:64, qsl[0], qsl[1]])
                nc.vector.tensor_copy(qB[64:96], qs_pat[0][96:128, qsl[0], qsl[1]])
                return (kwA, kwB), (qwA, qwB)

            def emit_xres(w):
                wrl, wc = w // 16, w % 16
                c0 = WS * wc
                xres = a_sb.tile([128, 360], F32, tag="xres", name=f"xres{w}", bufs=3)
                for qc in range(2):
                    rq = WS * wrl + 8 * qc
                    nc.gpsimd.dma_start(
                        xres[:, 180 * qc : 180 * qc + 180],
                        xr_pat[rq : rq + 8, c0 : c0 + WS, :],
                    )
                return xres

            def av_group(pv, h):
                """one contiguous 5-matmul PSUM accumulation group for head h
                of the previous window (groups must never interleave)."""
                colp, colf = 32 * (h % 4), 256 * (h // 4)
                for j in range(5):
                    kc = CHUNK_KC[j]
                    nc.tensor.matmul(
                        pv["av"][colp : colp + 32, colf : colf + 256],
                        pv["vw"][j][0:kc, 32 * h : 32 * h + 32],
                        pv["es"][j][0:kc, STC[h] : STC[h] + 256],
                        start=(j == 0),
                        stop=(j == 4),
                        tile_position=(0, colp),
                    )

            def filler(pv, j):
                """previous-window work dropped into round j's exp shadow"""
                if pv is None:
                    return
                w = pv["w"]
                if j == 0:
                    pv["av"] = a_av.tile([128, 512], F32, tag="av", name=f"av{w}")
                    av_group(pv, 0)
                    av_group(pv, 1)
                    av_group(pv, 2)
                elif j == 1:
                    av_group(pv, 3)
                    av_group(pv, 4)
                    av_group(pv, 5)
                    rsb = a_sb.tile([128, 512], F32R, tag="rsb")
                    nc.vector.tensor_copy(rsb[:], pv["av"][:])
                    pv["rsb"] = rsb
                elif j == 2:
                    ex = a_xp.tile([128, 512], F32, tag="xp", name=f"ex{w}")
                    nc.tensor.matmul(
                        ex[:, 0:256], E128[:], pv["rsb"][:, 0:256],
                        start=True, stop=True,
                    )
                    nc.tensor.matmul(
                        ex[:, 256:512], E64[:], pv["rsb"][0:64, 256:512],
                        start=True, stop=True,
                    )
                    exc = a_sb.tile([128, 512], F32, tag="exc")
                    nc.vector.tensor_copy(exc[:], ex[:])
                    ex_sb = a_sb.tile([128, 512], F32, tag="exsb")
                    nc.vector.reciprocal_approx_fast(ex_sb[:], exc[:])
                    att = a_sb.tile([128, 512], BF16, tag="att")
                    nc.vector.tensor_tensor(
                        att[:], pv["av"][:], ex_sb[:], op=mybir.AluOpType.mult
                    )
                    pv["att"] = att
                elif j == 3:
                    wrl, wc = w // 16, w % 16
                    c0 = WS * wc
                    att = pv["att"]
                    pp = a_xp.tile([128, 512], F32, tag="xp", name=f"pp{w}")
                    for qc in range(2):
                        nc.tensor.matmul(
                            pp[:, 180 * qc : 180 * qc + 180],
                            att[:, 128 * qc : 128 * (qc + 1)],
                            WP0[:],
                            start=True,
                            stop=False,
                        )
                        nc.tensor.matmul(
                            pp[:, 180 * qc : 180 * qc + 180],
                            att[0:64, 256 + 128 * qc : 256 + 128 * (qc + 1)],
                            WP1[:],
                            start=False,
                            stop=True,
                        )
                    ot = a_sb.tile([128, 360], F32, tag="ot")
                    nc.vector.tensor_tensor(
                        ot[:], pp[:, 0:360], pv["xres"][:], op=mybir.AluOpType.add
                    )
                    for qc in range(2):
                        rq = WS * wrl + 8 * qc
                        nc.sync.dma_start(
                            out_pat[rq : rq + 8, c0 : c0 + WS, :],
                            ot[:, 180 * qc : 180 * qc + 180],
                        )

            vw = emit_vw(0)
            kws, qws = emit_kw(0)
            prev = None
            for w in range(NWIN):
                wrl, wc = w // 16, w % 16
                r0, c0 = WS * wrl, WS * wc
                xres = emit_xres(w)
                # prefetch next window's k/q tiles and v chunks FIRST so the
                # DVE copies land a full window before S^T(w+1) reads them
                if w + 1 < NWIN:
                    nvw = emit_vw(w + 1)
                    nkws, nqws = emit_kw(w + 1)

                es_list = []
                for j in range(5):
                    kc = CHUNK_KC[j]
                    st = a_st.tile([128, 1536], F32, tag="st", name=f"st{w}_{j}")
                    for h in range(NH):
                        ktile = kws[0] if h < 3 else kws[1]
                        qtile = qws[0] if h < 3 else qws[1]
                        hr = HROW[h]
                        nc.tensor.matmul(
                            st[0 : kc, STC[h] : STC[h] + 256],
                            ktile[hr : hr + 32, 120 * j : 120 * j + kc],
                            qtile[hr : hr + 32, :],
                            start=True,
                            stop=True,
                            tile_position=(hr, 0),
                        )
                    es = a_es.tile([128, 1536], BF16, tag="es", name=f"es{w}_{j}")
                    nc.scalar.activation(
                        es[0:kc, :], st[0:kc, :], mybir.ActivationFunctionType.Exp
                    )
                    es_list.append(es)
                    filler(prev, j)

                prev = {"w": w, "es": es_list, "vw": vw, "xres": xres}
                if w + 1 < NWIN:
                    vw = nvw
                    kws, qws = nkws, nqws

            for j in range(5):
                filler(prev, j)

    nc.compile()
    return nc


def _prep_host(inputs):
    x = np.ascontiguousarray(inputs["x"], dtype=np.float32)[0]  # [65536, 180]
    norm_w = np.asarray(inputs["norm_w"], np.float32)
    norm_b = np.asarray(inputs["norm_b"], np.float32)
    q_w = np.asarray(inputs["q_w"], np.float32)
    q_b = np.asarray(inputs["q_b"], np.float32)
    kv_w = np.asarray(inputs["kv_w"], np.float32)
    kv_b = np.asarray(inputs["kv_b"], np.float32)
    proj_w = np.asarray(inputs["proj_w"], np.float32)
    proj_b = np.asarray(inputs["proj_b"], np.float32)

    scale = HD ** -0.5
    Wq = norm_w[:, None] * q_w * scale
    bq = (norm_b @ q_w + q_b) * scale
    Wk = norm_w[:, None] * kv_w[:, :C]
    bk = norm_b @ kv_w[:, :C] + kv_b[:C]
    Wv = norm_w[:, None] * kv_w[:, C:]
    bv = norm_b @ kv_w[:, C:] + kv_b[C:]

    # wqk [181, 512]: 4 M-chunks (q h0-3 | q h4-5 | k h0-3 | k h4-5), 32-col head blocks
    wqk = np.zeros((181, 512), np.float32)
    for h in range(NH):
        mc = 0 if h < 4 else 1
        col = 128 * mc + 32 * (h % 4)
        wqk[:C, col : col + HD] = Wq[:, HD * h : HD * (h + 1)]
        wqk[C, col : col + HD] = bq[HD * h : HD * (h + 1)]
        colk = 256 + col
        wqk[:C, colk : colk + HD] = Wk[:, HD * h : HD * (h + 1)]
        wqk[C, colk : colk + HD] = bk[HD * h : HD * (h + 1)]

    # wv [181, 192]: 32-col head blocks; cols 30/31 of each block = ones (bias row)
    wv = np.zeros((181, 192), np.float32)
    for h in range(NH):
        col = 32 * h
        wv[:C, col : col + HD] = Wv[:, HD * h : HD * (h + 1)]
        wv[C, col : col + HD] = bv[HD * h : HD * (h + 1)]
        wv[C, col + 30] = 1.0
        wv[C, col + 31] = 1.0

    # wp [192, 180]: head-padded proj rows
    wp = np.zeros((192, C), np.float32)
    for h in range(NH):
        row = 32 * (h % 4) if h < 4 else 128 + 32 * (h - 4)
        wp[row : row + HD, :] = proj_w[HD * h : HD * (h + 1), :]

    ident = np.eye(128, dtype=bfnp)
    e128 = np.zeros((128, 128), np.float32)
    for j in range(4):
        e128[32 * j + 30, 32 * j : 32 * j + 32] = 1.0
    e64 = np.zeros((64, 128), np.float32)
    for j in range(2):
        e64[32 * j + 30, 32 * j : 32 * j + 32] = 1.0
        e64[32 * j + 30, 64 + 32 * j : 64 + 32 * j + 32] = 1.0

    # per-core slabs
    xg = x.reshape(H, W, C)
    xpad = np.zeros((H + 2 * PADW, CS, C), np.float32)
    xpad[PADW : PADW + H, PADW : PADW + W, :] = xg
    xres_full = x + proj_b  # residual (+proj bias folded in)

    in_maps = []
    for c in range(NCORES):
        slab = np.zeros((T, C), np.float32)
        slab[:TS_REAL] = xpad[32 * c : 32 * c + RS].reshape(TS_REAL, C)
        xr = np.ascontiguousarray(
            xres_full[8192 * c : 8192 * (c + 1)], dtype=np.float32
        )
        in_maps.append(
            {
                "xs": slab,
                "xr": xr,
                "wqk": wqk.astype(bfnp),
                "wv": wv.astype(bfnp),
                "wp": wp.astype(bfnp),
                "ident": ident,
                "e128": e128,
                "e64": e64,
                "ones": np.ones((1, T), bfnp),
            }
        )
    return in_maps


def kernel(**inputs):
    global _CACHED, LAST_RESULTS
    if _CACHED is None:
        _CACHED = _build_program()
    nc = _CACHED
    in_maps = _prep_host(inputs)
    res = run_bass_kernel_spmd(
        nc,
        in_maps,
        list(range(NCORES)),
        trace=False,
    )
    LAST_RESULTS = res
    out = np.empty((1, H * W, C), np.float32)
    for c in range(NCORES):
        out[0, 8192 * c : 8192 * (c + 1), :] = res.results[c]["out"]
    return out
